# revision 10
# baseline (speedup 1.0000x reference)
"""Trainium2 Bass kernel for nn_ColorTransform: per-pixel degree-3 polynomial
color transform  y[b,c,h,w] = bias[c] + sum_f weight[f,c] * mono_f(x[b,:,h,w]).

Strategy (pure data parallel over batch across 8 cores; identical SPMD program):

The 3->19->3 per-pixel map is represented with R=8 affine forms
L_i = a_i.x + b_i (solved at runtime by Levenberg-Marquardt from a
precomputed init) such that

    y_c = sum_i cq[i,c] * L_i^3 + cs[i,c] * L_i^2       (exactly)

R=8 gives 16 groups of pixels per compute column-chunk (8 form-rows per
group = 128 partitions, the full machine width), twice the pixel density
of the generic linear-basis R=10 construction.

On-chip pipeline per compute unit (batch b, 1024 pixel columns, 16 groups):
  PE  M1  -> P1 = wm1_b^T @ X        [128, 1024] PSUM   (f16, 2x N=512)
  ACT     -> S = Square(P1)  (f32r)  SBUF
  DVE     -> Q = S * P1      (f32r)  SBUF
  PE  M2  -> P2 = Wq^T @ Q + Ws^T @ S  (f32r, accumulate) [48, 512]x2 PSUM
             at PSUM base partition 64*b  (two units packed per PSUM tile)
  ACT/DVE -> O copy-out fp32->fp16 [112, 1024] in ONE stream per unit-pair
DMA layout: X spans 97 partitions (1 ones row + 2*48), O spans 112
partitions, so DMA runs near full port bandwidth (the old layout used 36
partitions = ~30% of DMA ports).  Output is fp16 (cast on host afterwards).
"""
import os
import numpy as np
from math import factorial
from itertools import product as _product

import concourse.bass as bass
import concourse.tile as tile
from concourse import bacc, mybir
from concourse.bass_utils import run_bass_kernel_spmd

# ---------------------------------------------------------------- constants
B, C, H, W = 16, 3, 512, 512
HW = H * W
NCORES = 8
BPC = B // NCORES          # batches per core = 2
R = 8                      # affine forms per pixel-group
G = 16                     # pixel groups per batch per chunk (R*G = 128)
ND = 8192                  # pixel columns per group
NCHUNK = HW // (G * ND)    # 2 chunks per core
NCMP = 1024                # compute columns per unit
KX = 3 * G * BPC + 1       # 97 X-tile rows (1 ones row)
OROWS = 3 * G              # 48 output rows per unit
COPY_ENGINES = os.environ.get("COPY_ENGINES", "AADAADAD")  # per-q copy engines
ABL_NO_MUL = bool(int(os.environ.get("ABL_NO_MUL", "0")))   # Q aliases S (DVE idle)
ABL_TINY_OUT = bool(int(os.environ.get("ABL_TINY_OUT", "0")))  # 1-partition out-DMA
ABL_TINY_IN = bool(int(os.environ.get("ABL_TINY_IN", "0")))    # 1-row in-DMA

# LM init: forms P0 [R,4] (a0,a1,a2,b) and coeffs C0 [2R,3] (cubes then
# squares) solved offline for the reference (weight, bias); runtime LM
# re-solves from this init so the kernel is exact for the actual inputs.
INIT_P = np.array([
    [-0.1458396710740527, 1.2807526966324396, -1.1691583287784595, -1.5988501018144772],
    [0.8644570752490239, 3.6773383721315573, 0.882771159768557, 1.1584703163432764],
    [-1.4769139646370815, 0.6191088208591676, -1.0112044304815362, 0.24327677340046833],
    [0.5033846416824326, 0.7253125021307937, 0.5547977640973702, -0.9431034774566188],
    [-1.3986420366068164, -0.29731094403616554, 0.6460200919440608, -0.4957011526373761],
    [-0.8772441534293239, 0.07096541030582262, 1.2605136269128918, 0.11668293226917481],
    [0.025070919268324658, 2.4241108578006, 0.7614409333547284, -0.7386497541213923],
    [0.45117175263549203, -0.5107065561999109, 1.3676925338678338, -0.5863366274185755]])
INIT_C = np.array([
    [-0.16717900629543184, -0.24543631917095965, -0.2776440272958058],
    [-0.03690393025339779, 0.05795151728215451, 0.015339254807738102],
    [-0.4579245818025371, -0.3772054023798401, 0.013003119534341852],
    [0.09567397016650375, -0.9911119524856987, -0.2359834575786191],
    [-0.43098009187631225, 0.28707425418917465, 0.34499951003023166],
    [0.03719059929253637, -0.4934184292468183, -0.2855875363763469],
    [0.10606283690465622, -0.09163326739986088, 0.04337576937834181],
    [-0.7145891918484916, -0.7326026006653876, -0.23074163076194273],
    [-0.7297007107896267, -0.8533005112866605, -0.603621150494936],
    [0.0835133829328389, -0.22060207385037742, -0.008546110962447772],
    [-0.283285789268838, -0.1944903464509654, -0.04155398526307385],
    [0.1752845863410076, -0.8600910922825735, -0.13329074307574224],
    [-0.2968503172934026, -0.43330104037443995, 0.16725535102472738],
    [-0.6710621155738818, 0.4306079690803548, 0.08267154719230185],
    [0.37768052623537146, -0.4183455057720263, -0.2122034220173909],
    [-0.8261102222850351, -1.0823639987197975, -0.25596055629297876]])

MONO = [(0, 0, 0),
        (1, 0, 0), (0, 1, 0), (0, 0, 1),
        (2, 0, 0), (1, 1, 0), (1, 0, 1), (0, 2, 0), (0, 1, 1), (0, 0, 2),
        (3, 0, 0), (2, 1, 0), (2, 0, 1), (1, 2, 0), (1, 1, 1), (1, 0, 2),
        (0, 3, 0), (0, 2, 1), (0, 1, 2), (0, 0, 3)]


def _term_table(power):
    K, MU, IDX = [], [], []
    for ks in _product(range(power + 1), repeat=4):
        if sum(ks) != power:
            continue
        k0, k1, k2, kb = ks
        m = factorial(power) / (factorial(k0) * factorial(k1) * factorial(k2) * factorial(kb))
        K.append([k0, k1, k2, kb]); MU.append(m); IDX.append(MONO.index((k0, k1, k2)))
    return np.array(K, np.float64), np.array(MU), np.array(IDX)


_K3, _MU3, _IDX3 = _term_table(3)
_K2, _MU2, _IDX2 = _term_table(2)


def _basisA(P, K, MU, IDX):
    prods = np.prod(P[None, :, :] ** K[:, None, :], axis=2)
    A = np.zeros((20, P.shape[0]))
    np.add.at(A, IDX, MU[:, None] * prods)
    return A


def _dbasisA(P, K, MU, IDX):
    out = []
    for dd in range(4):
        Kd = K.copy(); coef = K[:, dd].copy(); Kd[:, dd] = np.maximum(K[:, dd] - 1, 0)
        prods = np.prod(P[None, :, :] ** Kd[:, None, :], axis=2) * coef[:, None]
        Ad = np.zeros((20, P.shape[0])); np.add.at(Ad, IDX, MU[:, None] * prods)
        out.append(Ad)
    return out


def _full_basis(P):
    return np.concatenate([_basisA(P, _K3, _MU3, _IDX3),
                           _basisA(P, _K2, _MU2, _IDX2)], axis=1)


def _lm(P, Cc, T, iters=300):
    x = np.concatenate([P.ravel(), Cc.ravel()]); n4 = 4 * R

    def unpack(x):
        return x[:n4].reshape(R, 4), x[n4:].reshape(2 * R, 3)

    def res(x):
        P, Cc = unpack(x)
        return (_full_basis(P) @ Cc - T).ravel()

    def jac(x):
        P, Cc = unpack(x)
        A3 = _basisA(P, _K3, _MU3, _IDX3); A2 = _basisA(P, _K2, _MU2, _IDX2)
        dA3 = _dbasisA(P, _K3, _MU3, _IDX3); dA2 = _dbasisA(P, _K2, _MU2, _IDX2)
        J = np.zeros((60, x.size))
        for i in range(R):
            for dd in range(4):
                blk = dA3[dd][:, i][:, None] * Cc[i][None, :] + \
                      dA2[dd][:, i][:, None] * Cc[R + i][None, :]
                J[:, i * 4 + dd] = blk.ravel()
        for i in range(2 * R):
            col = A3[:, i] if i < R else A2[:, i - R]
            for c in range(3):
                blk = np.zeros((20, 3)); blk[:, c] = col
                J[:, n4 + i * 3 + c] = blk.ravel()
        return J

    r = res(x); cost = r @ r; lam = 1e-6
    for _ in range(iters):
        J = jac(x); JtJ = J.T @ J; Jtr = J.T @ r; ok = False
        for _try in range(60):
            try:
                dx = np.linalg.solve(JtJ + lam * np.diag(np.maximum(np.diag(JtJ), 1e-10)), -Jtr)
            except np.linalg.LinAlgError:
                lam *= 10; continue
            xn = x + dx; rn = res(xn); cn = rn @ rn
            if cn < cost:
                x, r, cost = xn, rn, cn; lam = max(lam * 0.3, 1e-16); ok = True; break
            lam *= 4.0
        if not ok or cost < 1e-28:
            break
    P, Cc = unpack(x)
    return P, Cc, np.sqrt(cost)


def _solve_forms(weight, bias):
    """-> P16 [R,4] (fp16 grid), C [2R,3] float64: exact decomposition of the
    target polynomial into cubes+squares of affine forms."""
    T = np.zeros((20, 3))
    T[0] = np.asarray(bias, np.float64)
    T[1:] = np.asarray(weight, np.float64)
    P, Cc, resv = _lm(INIT_P.copy(), INIT_C.copy(), T)
    if resv > 1e-8:   # unexpected weights: retry from random inits
        rng = np.random.default_rng(0)
        for _ in range(60):
            P0 = rng.normal(size=(R, 4)) * 0.8
            C0 = np.linalg.lstsq(_full_basis(P0), T, rcond=None)[0]
            P1_, C1_, r1 = _lm(P0, C0, T, iters=400)
            if r1 < resv:
                P, Cc, resv = P1_, C1_, r1
            if resv < 1e-10:
                break
    # quantize forms to the fp16 grid the hardware will see, re-fit coeffs
    P16 = P.astype(np.float16).astype(np.float64)
    C16 = np.linalg.lstsq(_full_basis(P16), T, rcond=None)[0]
    return P16, C16


# ---------------------------------------------------------------- weights
def _make_weights(P16, C16):
    av = P16[:, :3]; bv = P16[:, 3]
    cq = C16[:R]; cs = C16[R:]
    # M1: X rows 0=ones, 1 + b*3G + v*G + g ; P1 rows g*R + i
    wm1 = np.zeros((BPC, KX, R * G), np.float32)
    for b in range(BPC):
        for g in range(G):
            for i in range(R):
                col = g * R + i
                wm1[b, 0, col] = bv[i]
                for v in range(C):
                    wm1[b, 1 + b * 3 * G + v * G + g, col] = av[i, v]
    # M2: contraction rows g*R+i -> out col c*G + g (padded to 64 cols so the
    # matmul defines the full PSUM partition range it occupies)
    w2q = np.zeros((R * G, 64), np.float32)
    w2s = np.zeros((R * G, 64), np.float32)
    for g in range(G):
        for i in range(R):
            for c in range(3):
                w2q[g * R + i, c * G + g] = cq[i, c]
                w2s[g * R + i, c * G + g] = cs[i, c]
    return wm1.astype(np.float16), w2q.astype(np.float16), w2s.astype(np.float16)


# ---------------------------------------------------------------- bass build
_NC_CACHE = {}


def build_nc(reps=1):
    if reps in _NC_CACHE:
        return _NC_CACHE[reps]
    f32, f16, f32r = mybir.dt.float32, mybir.dt.float16, mybir.dt.float32r
    nc = bacc.Bacc("TRN2", target_bir_lowering=False, debug=False, num_devices=NCORES)

    xs = nc.dram_tensor("xs", [BPC, C, HW], f16, kind="ExternalInput")
    wm1d = [nc.dram_tensor(f"wm1_{b}", [KX, R * G], f16, kind="ExternalInput")
            for b in range(BPC)]
    w2qd = nc.dram_tensor("w2q", [R * G, 64], f16, kind="ExternalInput")
    w2sd = nc.dram_tensor("w2s", [R * G, 64], f16, kind="ExternalInput")
    y = nc.dram_tensor("y", [BPC, NCHUNK, OROWS, ND], f16, kind="ExternalOutput")

    with tile.TileContext(nc) as tc:
        with (
            tc.tile_pool(name="wpool", bufs=1) as wpool,
            tc.tile_pool(name="xpool", bufs=2) as xpool,
            tc.tile_pool(name="spool", bufs=3) as spool,
            tc.tile_pool(name="qpool", bufs=3) as qpool,
            tc.tile_pool(name="opool", bufs=2) as opool,
            tc.tile_pool(name="p1pool", bufs=2, space="PSUM") as p1pool,
            tc.tile_pool(name="p2pool", bufs=2, space="PSUM") as p2pool,
        ):
            wm1_sb = []
            for b in range(BPC):
                t = wpool.tile([KX, R * G], f16, tag=f"wm1_{b}")
                nc.sync.dma_start(t[:], wm1d[b][:])
                wm1_sb.append(t)

            def load_w2(name, dram):
                t = wpool.tile([R * G, 64], f16, tag=name)
                nc.sync.dma_start(t[:], dram[:])
                return t

            w2q_r = load_w2("w2q", w2qd)
            w2s_r = load_w2("w2s", w2sd)

            # ones row in both X buffers
            for _ in range(2):
                xt0 = xpool.tile([KX, ND], f16, tag="X")
                nc.gpsimd.memset(xt0[0:1, :], 1.0)

            units = [(k, q, b) for k in range(NCHUNK) for q in range(ND // NCMP)
                     for b in range(BPC)]

            def body():
                xt_state, o_state, p2_state = {}, {}, {}
                # chunk DMAs up front for maximal prefetch lead
                for k in range(NCHUNK):
                    lo = k * G * ND
                    xt = xpool.tile([KX, ND], f16, tag="X", name=f"xt{k}")
                    if ABL_TINY_IN:
                        nc.sync.dma_start(xt[1:2], xs[0, 0:1, lo:lo + ND])
                    else:
                        nc.sync.dma_start(
                            xt[1:97],
                            xs[:, :, lo:lo + G * ND].rearrange(
                                "b v (g n) -> b v g n", n=ND))
                    xt_state[k] = xt
                    o_state[k] = opool.tile([112, ND], f16, tag="O", name=f"o{k}")

                def stage1(k, q, b):
                    xt = xt_state[k]
                    p1 = p1pool.tile([R * G, NCMP], f32, tag="P1")
                    cl = q * NCMP
                    for h in range(NCMP // 512):
                        nc.tensor.matmul(p1[:, h * 512:(h + 1) * 512], wm1_sb[b][:],
                                         xt[:, cl + h * 512:cl + (h + 1) * 512],
                                         start=True, stop=True)
                    return p1

                def stage2(p1):
                    s = spool.tile([R * G, NCMP], f16, tag="S")
                    nc.scalar.square(s[:], p1[:])
                    if ABL_NO_MUL:
                        return s, s
                    qq = qpool.tile([R * G, NCMP], f16, tag="Q")
                    nc.vector.tensor_mul(qq[:], s[:], p1[:])
                    return s, qq

                def stage3(k, q, b, s, qq):
                    if b == 0:
                        p2_state[(k, q)] = p2pool.tile([128, NCMP], f32, tag="P2", name=f"p2_{k}_{q}")
                    p2 = p2_state[(k, q)]
                    ob = 64 * b
                    for h in range(NCMP // 512):
                        hl, hh = h * 512, (h + 1) * 512
                        nc.tensor.matmul(p2[ob:ob + 64, hl:hh], w2q_r[:],
                                         qq[:, hl:hh], start=True, stop=False)
                        nc.tensor.matmul(p2[ob:ob + 64, hl:hh], w2s_r[:],
                                         s[:, hl:hh], start=False, stop=True)
                    if b == BPC - 1:
                        o = o_state[k]
                        cl = q * NCMP
                        if COPY_ENGINES[q % len(COPY_ENGINES)] == "A":
                            nc.scalar.copy(o[0:112, cl:cl + NCMP], p2[0:112, :])
                        else:
                            nc.vector.tensor_copy(o[0:112, cl:cl + NCMP], p2[0:112, :])
                        if q == ND // NCMP - 1:
                            for bb in range(BPC):
                                if ABL_TINY_OUT:
                                    nc.sync.dma_start(y[bb, k, 0:1],
                                                      o[64 * bb:64 * bb + 1])
                                else:
                                    nc.sync.dma_start(y[bb, k],
                                                      o[64 * bb:64 * bb + OROWS])

                # software pipeline: stage1(u) ahead of stage2(u-1) ahead of
                # stage3(u-2) so no engine queue head-of-line blocks.
                q2, q3 = [], []

                def pump(force=False):
                    if q2 and (force or len(q2) > 1):
                        (k2, qq2, b2, p12) = q2.pop(0)
                        s_t, qq_t = stage2(p12)
                        q3.append((k2, qq2, b2, s_t, qq_t))
                    if q3 and (force or len(q3) > 1):
                        stage3(*q3.pop(0))

                for (k, q, b) in units:
                    p1 = stage1(k, q, b)
                    q2.append((k, q, b, p1))
                    pump()
                while q2 or q3:
                    pump(force=True)

            if reps == 1:
                body()
            else:
                hint = (mybir.EngineType.PE, mybir.EngineType.Activation,
                        mybir.EngineType.DVE, mybir.EngineType.SP)
                un = 1
                for cand in (4, 2):
                    if reps % cand == 0:
                        un = cand
                        break
                with tc.For_i(0, reps // un, 1, hint_engines=hint):
                    for _ in range(un):
                        body()

    nc.compile()
    _NC_CACHE[reps] = nc
    return nc


def make_in_maps(x, weight, bias):
    P16, C16 = _solve_forms(weight, bias)
    wm1, w2q, w2s = _make_weights(P16, C16)
    shared = {"w2q": w2q, "w2s": w2s}
    for b in range(BPC):
        shared[f"wm1_{b}"] = np.ascontiguousarray(wm1[b])
    x = np.ascontiguousarray(np.asarray(x, np.float16)).reshape(B, C, HW)
    return [dict(shared, xs=x[i * BPC:(i + 1) * BPC]) for i in range(NCORES)]


def kernel(x, weight, bias, degree=3, **_unused):
    assert int(degree) == 3, "kernel specialized for degree=3"
    nc = build_nc(reps=1)
    in_maps = make_in_maps(x, weight, bias)
    res = run_bass_kernel_spmd(nc, in_maps, core_ids=list(range(NCORES)))
    out = np.empty((B, C, HW), np.float32)
    for i in range(NCORES):
        # y layout [BPC, NCHUNK, c*G+g, ND] -> [BPC, C, HW]
        yv = res.results[i]["y"].astype(np.float32)
        yv = yv.reshape(BPC, NCHUNK, C, G, ND).transpose(0, 2, 1, 3, 4)
        out[i * BPC:(i + 1) * BPC] = yv.reshape(BPC, C, HW)
    return out.reshape(B, C, H, W)


if __name__ == "__main__":
    rng = np.random.default_rng(0)
    x = rng.uniform(0, 1, size=(B, C, H, W)).astype(np.float32)
    weight = rng.normal(size=(19, 3)).astype(np.float32)
    bias = rng.normal(size=(3,)).astype(np.float32)
    got = kernel(x, weight, bias, 3)
    print("ran; out shape", got.shape)
